# revision 1
# baseline (speedup 1.0000x reference)
"""Trainium2 Bass kernel for nn_KinematicLayer: batched forward kinematics.

Full inputs x:[524288,26] f32 -> out:[524288,51] f32.
Data-parallel across 8 NeuronCores (65536 samples/core), 2 chunks/core of
[128 partitions x 256 samples].  Per-sample state tracked as (R 3x3, t 3)
instead of 4x4 homogeneous matmuls; the five limb chains (neck+head, 2 legs,
2 arms) share one instruction stream batched along the free dim (FD=1280).
Trig via half-angle identities keeps every ACT Sin argument inside the
spline's valid [-pi,pi] range:  u=sin(x/2), w=sin(x/4), v=1-2w^2=cos(x/2),
cos=1-2u^2, sin=2uv.  Intermediates are fp16 (DVE 2x mode), outputs fp32.
"""
import numpy as np
import concourse.bass as bass
import concourse.tile as tile
from concourse import bacc, mybir
from concourse.bass_utils import run_bass_kernel_spmd

AF = mybir.ActivationFunctionType
ALU = mybir.AluOpType
f32, f16 = mybir.dt.float32, mybir.dt.float16

N, K, J = 524288, 26, 51
NCORE = 8
NPC = N // NCORE            # 65536 samples per core
FD = 256                    # samples per partition per chunk
CHUNK = 128 * FD            # 32768 samples per chunk
NCHUNK = NPC // CHUNK       # 2

_S = np.array([300.0, 350.0, 75.0, 400.0, 73.96, 249.03, 250.0, 250.0, 170.0],
              np.float32) / 300.0
S0, S1, S2, S3, S4, S5, S6, S7, S8 = [float(v) for v in _S]

# chain order: (neck, Lleg, Rleg, Larm, Rarm); euler angle bases 5,9,13,17,21
# knee-level joints (2,5,8,11,14), distal joints (3,6,9,12,15): both step 3.
DT1 = [S4, -S1, -S1, -S7, -S7]   # signed first-translation lengths
DT2 = [S5, -S0, -S0, -S6, -S6]   # signed distal-translation lengths

FDC = 5 * FD                 # batched chain free dim


def ap_of(t):
    return t[:]


def mk(ap, off, dims):
    """Custom free-dim AP on the same tile/tensor (keeps partition dim)."""
    return bass.AP(ap.tensor, ap.offset + off, [list(ap.ap[0])] + dims)


def build():
    nc = bacc.Bacc("TRN2", target_bir_lowering=False, debug=False,
                   num_devices=NCORE)
    x = nc.dram_tensor("x", [NPC, K], f32, kind="ExternalInput").ap()
    y = nc.dram_tensor("y", [NPC, J], f32, kind="ExternalOutput").ap()

    with tile.TileContext(nc) as tc:
        with (
            tc.tile_pool(name="io", bufs=1) as io,       # X, Y double buffered
            tc.tile_pool(name="per", bufs=1) as per,     # persistent per chunk
            tc.tile_pool(name="scr", bufs=1) as scr,     # small scratch
        ):
            for ch in range(NCHUNK):
                build_chunk(nc, tc, io, per, scr, x, y, ch)
    nc.compile()
    return nc


def build_chunk(nc, tc, io, per, scr, x, y, ch):
    V, A = nc.vector, nc.scalar
    base = ch * CHUNK

    X = io.tile([128, K * FD], f32, tag="X")
    HX = K * FD // 2
    for h in range(2):
        nc.gpsimd.dma_start(X[:, h * HX:(h + 1) * HX],
                            bass.AP(x.tensor, base * K + h * HX,
                                    [[FD * K, 128], [1, HX]]))
    Y = io.tile([128, J * FD], f32, tag="Y")
    Xa = X[:]
    Ya = Y[:]

    def xang(k):                       # angle k strided view [128, FD]
        return mk(Xa, k, [[K, FD]])

    def ycol(c):                       # output scalar col c (0..50) strided
        return mk(Ya, c, [[J, FD]])

    def ygrp(c0):                      # batched 5-chain joint write, offset c0
        return mk(Ya, c0, [[9, 5], [J, FD]])

    # ---------------- trig: 6 groups ----------------
    # group APs reading X: pelvis/torso = angles 0..4 step 1; chain pos j =
    # angles 5+j step 4 across chains.
    def trig(tag, xap, n):
        fd = n * FD
        u = scr.tile([128, fd], f16, tag="trigU", name="trigU")
        w = scr.tile([128, fd], f16, tag="trigW", name="trigW")
        A.activation(u[:], xap, AF.Sin, scale=0.5)
        A.activation(w[:], xap, AF.Sin, scale=0.25)
        q = scr.tile([128, fd], f16, tag="trigQ", name="trigQ")
        c = per.tile([128, fd], f16, tag=f"C{tag}", name=f"C{tag}")
        s = per.tile([128, fd], f16, tag=f"S{tag}", name=f"S{tag}")
        A.square(q[:], u[:])
        V.tensor_scalar(c[:], q[:], -2.0, 1.0, ALU.mult, ALU.add)
        A.square(q[:], w[:])
        V.tensor_scalar(q[:], q[:], -2.0, 1.0, ALU.mult, ALU.add)  # v in q
        V.scalar_tensor_tensor(s[:], u[:], 2.0, q[:], ALU.mult, ALU.mult)
        return c, s

    Cpt, Spt = trig("pt", mk(Xa, 0, [[1, 5], [K, FD]]), 5)
    CS = [trig(f"p{j}", mk(Xa, 5 + j, [[4, 5], [K, FD]]), 5) for j in range(4)]

    def pt(t, i):                      # pelvis/torso angle slice i of 0..4
        return t[:, i * FD:(i + 1) * FD]

    c0, s0 = pt(Cpt, 0), pt(Spt, 0)
    c1, s1 = pt(Cpt, 1), pt(Spt, 1)
    c2, s2 = pt(Cpt, 2), pt(Spt, 2)
    c3, s3 = pt(Cpt, 3), pt(Spt, 3)
    c4, s4 = pt(Cpt, 4), pt(Spt, 4)

    def tt(out, a, b, op):
        V.tensor_tensor(out, a, b, op)

    def fresh(tag, fd=FD, dt=f16, pool=None):
        return (pool or scr).tile([128, fd], dt, tag=tag, name=tag)

    def mul(a, b, tag="m", fd=FD):
        o = fresh(tag, fd=fd)
        tt(o[:], a, b, ALU.mult)
        return o[:]

    def nmul(a, b, tag="m"):           # -(a*b)
        o = fresh(tag)
        V.scalar_tensor_tensor(o[:], a, -1.0, b, ALU.mult, ALU.mult)
        return o[:]

    def comb(a, b, op, tag="m", pool=None, fd=FD):
        o = fresh(tag, fd=fd, pool=pool)
        tt(o[:], a, b, op)
        return o[:]

    # ---------------- pelvis R ----------------
    ms0s1 = mul(s0, s1, "ms01")
    mc0s1 = mul(c0, s1, "mc01")
    P1x = nmul(s0, c1, "P1x")
    P1y = mul(c0, c1, "P1y")
    P1z = s1                                        # alias
    P0x = comb(mul(c0, c2), mul(ms0s1, s2, "m2"), ALU.subtract, "P0x", per)
    P0y = comb(mul(s0, c2), mul(mc0s1, s2, "m2"), ALU.add, "P0y", per)
    P0z = nmul(c1, s2, "P0z")
    P2x = comb(mul(c0, s2), mul(ms0s1, c2, "m2"), ALU.add, "P2x", per)
    P2y = comb(mul(s0, s2), mul(mc0s1, c2, "m2"), ALU.subtract, "P2y", per)
    P2z = mul(c1, c2, "P2z")
    P0 = (P0x, P0y, P0z)
    P1 = (P1x, P1y, P1z)
    P2 = (P2x, P2y, P2z)

    # ---------------- torso R = Rpel @ Rz3 @ Ry4 ----------------
    def colupd(cc, ss, A3, B3, tagp, pool=None, fd=FD):
        """returns cc*A + ss*B per component."""
        out = []
        for i, (a, b) in enumerate(zip(A3, B3)):
            out.append(comb(mul(cc, a, "ca", fd), mul(ss, b, "cb", fd), ALU.add,
                            f"{tagp}{i}", pool, fd))
        return tuple(out)

    def colupd_sub(cc, ss, A3, B3, tagp, pool=None, fd=FD):
        """returns cc*A - ss*B per component."""
        out = []
        for i, (a, b) in enumerate(zip(A3, B3)):
            out.append(comb(mul(cc, a, "ca", fd), mul(ss, b, "cb", fd), ALU.subtract,
                            f"{tagp}{i}", pool, fd))
        return tuple(out)

    D0t = colupd(c3, s3, P0, P1, "D0t")
    D1t = colupd_sub(c3, s3, P1, P0, "D1t", per)       # E1 = D1t
    E0 = colupd_sub(c4, s4, D0t, P2, "E0", per)
    E2 = colupd(s4, c4, D0t, P2, "E2", per)

    # ---------------- phase A translations ----------------
    scH = fresh("scH")
    V.tensor_copy(scH[:], mk(Xa, 25, [[K, FD]]))       # scale as fp16

    TP = [per.tile([128, FDC], f16, tag=f"TP{c}", name=f"TP{c}") for c in range(3)]

    def tp_slice(c, i):
        return TP[c][:, i * FD:(i + 1) * FD]

    for c in range(3):
        # torso t = S3*scale*D1 -> Y joint1 + TP[neck]
        V.scalar_tensor_tensor(ycol(3 * 1 + c), scH[:], S3, D1t[c],
                               ALU.mult, ALU.mult)
        V.scalar_tensor_tensor(tp_slice(c, 0), scH[:], S3, D1t[c],
                               ALU.mult, ALU.mult)
        # hips: +-S2*scale*P0 -> TP legs + Y joints 4,7
        V.scalar_tensor_tensor(tp_slice(c, 1), scH[:], S2, P0[c],
                               ALU.mult, ALU.mult)
        V.scalar_tensor_tensor(tp_slice(c, 2), scH[:], -S2, P0[c],
                               ALU.mult, ALU.mult)
        A.copy(ycol(3 * 4 + c), tp_slice(c, 1))
        A.copy(ycol(3 * 7 + c), tp_slice(c, 2))
        # shoulders: t_tor +- S8*scale*E0 -> TP arms + Y joints 10,13
        u = fresh("shu")
        V.scalar_tensor_tensor(u[:], scH[:], S8, E0[c], ALU.mult, ALU.mult)
        tt(tp_slice(c, 3), tp_slice(c, 0), u[:], ALU.add)
        tt(tp_slice(c, 4), tp_slice(c, 0), u[:], ALU.subtract)
        A.copy(ycol(3 * 10 + c), tp_slice(c, 3))
        A.copy(ycol(3 * 13 + c), tp_slice(c, 4))
    # pelvis joint 0 = 0
    V.memset(mk(Ya, 0, [[J, FD], [1, 3]]), 0.0)

    # ---------------- batched parent-R tiles ----------------
    # chains: 0=neck(E), 1,2=legs(P), 3,4=arms(E)
    PR = [[per.tile([128, FDC], f16, tag=f"PR{c}{i}", name=f"PR{c}{i}") for i in range(3)]
          for c in range(3)]
    for ci, (Ecol, Pcol) in enumerate(((E0, P0), (D1t, P1), (E2, P2))):
        for i in range(3):
            dst = PR[ci][i][:]
            e = Ecol[i]
            p = Pcol[i]
            def bc2(src):
                return bass.AP(src.tensor, src.offset,
                               [list(src.ap[0]), [0, 2], [1, FD]])
            A.copy(mk(dst, 0, [[1, FD]]), e)
            A.copy(mk(dst, FD, [[1, 2 * FD]]), bc2(p))
            A.copy(mk(dst, 3 * FD, [[1, 2 * FD]]), bc2(e))
    PR0, PR1, PR2 = PR

    def prc(c):
        return tuple(PR[c][i][:] for i in range(3))

    cA, sA = (t[:] for t in CS[0])
    cB, sB = (t[:] for t in CS[1])
    cG, sG = (t[:] for t in CS[2])
    cD, sD = (t[:] for t in CS[3])

    # ---------------- batched chain (FD=1280 ops) ----------------
    bD0 = colupd(cA, sA, prc(0), prc(1), "bD0", per, FDC)
    bD1 = colupd_sub(cA, sA, prc(1), prc(0), "bD1", per, FDC)
    bK1 = colupd(cB, sB, bD1, prc(2), "bK1", per, FDC)
    bK2 = colupd_sub(cB, sB, prc(2), bD1, "bK2", per, FDC)
    bK2p = colupd(sG, cG, bD0, bK2, "bD1", per, FDC)  # reuse bD1 slots
    bC1 = colupd(cD, sD, bK1, bK2p, "bD0", per, FDC)  # reuse bD0 slots

    # dT tiles: per-chain signed bone length * scale
    scB = fresh("scB", FDC)
    V.tensor_copy(scB[:], mk(scH[:], 0, [[0, 5], [1, FD]]))
    dT1 = fresh("dT1", FDC)
    dT2 = fresh("dT2", FDC)
    for i in range(5):
        sl = slice(i * FD, (i + 1) * FD)
        A.mul(dT1[:, sl], scB[:, sl], DT1[i])
        A.mul(dT2[:, sl], scB[:, sl], DT2[i])

    for c in range(3):
        u = fresh("btr", FDC)
        tt(u[:], dT1[:], bK1[c], ALU.mult)
        tt(ygrp(3 * 2 + c), TP[c][:], u[:], ALU.add)       # knee-level joints
        u2 = fresh("btr2", FDC)
        tt(u2[:], dT2[:], bC1[c], ALU.mult)
        tt(ygrp(3 * 3 + c), ygrp(3 * 2 + c), u2[:], ALU.add)  # distal joints

    # ---------------- thorax = 0.5*(p8 + p6) ----------------
    for c in range(3):
        h = fresh("thx")
        tt(h[:], ycol(3 * 8 + c), ycol(3 * 6 + c), ALU.add)
        A.mul(ycol(48 + c), h[:], 0.5)

    HY = J * FD // 2
    for h in range(2):
        nc.gpsimd.dma_start(bass.AP(y.tensor, base * J + h * HY,
                                    [[FD * J, 128], [1, HY]]),
                            Y[:, h * HY:(h + 1) * HY])


_NC = None


def kernel(x: np.ndarray) -> np.ndarray:
    global _NC
    if _NC is None:
        _NC = build()
    x = np.ascontiguousarray(x, dtype=np.float32)
    shards = x.reshape(NCORE, NPC, K)
    res = run_bass_kernel_spmd(
        _NC, [{"x": shards[i]} for i in range(NCORE)],
        core_ids=list(range(NCORE)))
    return np.concatenate([r["y"] for r in res.results], axis=0)



# revision 2
# speedup vs baseline: 2.6385x; 2.6385x over previous
"""Trainium2 Bass kernel for nn_KinematicLayer: batched forward kinematics.

Full inputs x:[524288,26] f32 -> out:[524288,51] f32.

End-to-end wall time is dominated by the axon host<->device tunnel
(~50-80 MB/s, serial), so the kernel minimizes wire bytes:
  - x is uploaded as f16 ([N,26], 27 MB instead of 55 MB);
  - the device returns only the 13 non-derivable joint positions as f16
    ([N,39], 41 MB instead of 107 MB).  Host derives the rest:
    p0 = 0, p7 = -p4, p13 = 2*p1 - p10, thorax = (p6+p8)/2.
  - the jitted executable is cached across calls, and the previous call's
    (already fetched) device output buffers are donated back as the
    custom-call output operands, so no zero-buffer upload per call.

Device compute (per core: 65536 samples, 2 chunks of 128x256): per-sample
state tracked as (R 3x3, t 3); the five limb chains share one instruction
stream batched along the free dim (FD=1280).  Trig via half-angle
identities keeps every ACT Sin argument inside the spline's valid
[-pi,pi] range: u=sin(x/2), w=sin(x/4), v=1-2w^2=cos(x/2), cos=1-2u^2,
sin=2uv.  Intermediates fp16 (DVE 2x mode).
"""
import numpy as np
import jax
from jax.sharding import Mesh, PartitionSpec
from jax.experimental.shard_map import shard_map

import concourse.bass as bass
import concourse.tile as tile
import concourse.bass2jax as b2j
from concourse import bacc, mybir

AF = mybir.ActivationFunctionType
ALU = mybir.AluOpType
f32, f16 = mybir.dt.float32, mybir.dt.float16

N, K = 524288, 26
J = 39                      # 13 joints x 3 shipped to host
NCORE = 8
NPC = N // NCORE            # 65536 samples per core
FD = 256                    # samples per partition per chunk
CHUNK = 128 * FD            # 32768 samples per chunk
NCHUNK = NPC // CHUNK       # 2

_S = np.array([300.0, 350.0, 75.0, 400.0, 73.96, 249.03, 250.0, 250.0, 170.0],
              np.float32) / 300.0
S0, S1, S2, S3, S4, S5, S6, S7, S8 = [float(v) for v in _S]

# chain order: (neck, Lleg, Rleg, Larm, Rarm); euler angle bases 5,9,13,17,21
DT1 = [S4, -S1, -S1, -S7, -S7]   # signed first-translation lengths
DT2 = [S5, -S0, -S0, -S6, -S6]   # signed distal-translation lengths

FDC = 5 * FD                 # batched chain free dim

# Output column layout (all f16):
#   cols  0..14 : knee-level joints (2,5,8,11,14), chain-major, 3 per chain
#   cols 15..29 : distal joints     (3,6,9,12,15), chain-major
#   cols 30..32 : joint 1 (torso)
#   cols 33..35 : joint 4 (left hip)
#   cols 36..38 : joint 10 (left shoulder)


def mk(ap, off, dims):
    """Custom free-dim AP on the same tile/tensor (keeps partition dim)."""
    return bass.AP(ap.tensor, ap.offset + off, [list(ap.ap[0])] + dims)


def build():
    nc = bacc.Bacc("TRN2", target_bir_lowering=False, debug=False,
                   num_devices=NCORE)
    x = nc.dram_tensor("x", [NPC, K], f16, kind="ExternalInput").ap()
    y = nc.dram_tensor("y", [NPC, J], f16, kind="ExternalOutput").ap()

    with tile.TileContext(nc) as tc:
        with (
            tc.tile_pool(name="io", bufs=1) as io,
            tc.tile_pool(name="per", bufs=1) as per,
            tc.tile_pool(name="scr", bufs=1) as scr,
        ):
            for ch in range(NCHUNK):
                build_chunk(nc, tc, io, per, scr, x, y, ch)
    nc.compile()
    return nc


def build_chunk(nc, tc, io, per, scr, x, y, ch):
    V, A = nc.vector, nc.scalar
    base = ch * CHUNK

    X = io.tile([128, K * FD], f16, tag="X")
    HX = K * FD // 2
    for h in range(2):
        nc.gpsimd.dma_start(X[:, h * HX:(h + 1) * HX],
                            bass.AP(x.tensor, base * K + h * HX,
                                    [[FD * K, 128], [1, HX]]))
    Y = io.tile([128, J * FD], f16, tag="Y")
    Xa = X[:]
    Ya = Y[:]

    def ycol(c):                       # output scalar col c strided [128,FD]
        return mk(Ya, c, [[J, FD]])

    def ygrp(c0):                      # batched 5-chain joint write
        return mk(Ya, c0, [[3, 5], [J, FD]])

    # ---------------- trig: 5 groups ----------------
    def trig(tag, xap, n):
        fd = n * FD
        u = scr.tile([128, fd], f16, tag="trigU", name="trigU")
        w = scr.tile([128, fd], f16, tag="trigW", name="trigW")
        A.activation(u[:], xap, AF.Sin, scale=0.5)
        A.activation(w[:], xap, AF.Sin, scale=0.25)
        q = scr.tile([128, fd], f16, tag="trigQ", name="trigQ")
        c = per.tile([128, fd], f16, tag=f"C{tag}", name=f"C{tag}")
        s = per.tile([128, fd], f16, tag=f"S{tag}", name=f"S{tag}")
        A.square(q[:], u[:])
        V.tensor_scalar(c[:], q[:], -2.0, 1.0, ALU.mult, ALU.add)
        A.square(q[:], w[:])
        V.tensor_scalar(q[:], q[:], -2.0, 1.0, ALU.mult, ALU.add)  # v in q
        V.scalar_tensor_tensor(s[:], u[:], 2.0, q[:], ALU.mult, ALU.mult)
        return c, s

    Cpt, Spt = trig("pt", mk(Xa, 0, [[1, 5], [K, FD]]), 5)
    CS = [trig(f"p{j}", mk(Xa, 5 + j, [[4, 5], [K, FD]]), 5) for j in range(4)]

    def pt(t, i):
        return t[:, i * FD:(i + 1) * FD]

    c0, s0 = pt(Cpt, 0), pt(Spt, 0)
    c1, s1 = pt(Cpt, 1), pt(Spt, 1)
    c2, s2 = pt(Cpt, 2), pt(Spt, 2)
    c3, s3 = pt(Cpt, 3), pt(Spt, 3)
    c4, s4 = pt(Cpt, 4), pt(Spt, 4)

    def tt(out, a, b, op):
        V.tensor_tensor(out, a, b, op)

    def fresh(tag, fd=FD, dt=f16, pool=None):
        return (pool or scr).tile([128, fd], dt, tag=tag, name=tag)

    def mul(a, b, tag="m", fd=FD):
        o = fresh(tag, fd=fd)
        tt(o[:], a, b, ALU.mult)
        return o[:]

    def nmul(a, b, tag="m"):           # -(a*b)
        o = fresh(tag)
        V.scalar_tensor_tensor(o[:], a, -1.0, b, ALU.mult, ALU.mult)
        return o[:]

    def comb(a, b, op, tag="m", pool=None, fd=FD):
        o = fresh(tag, fd=fd, pool=pool)
        tt(o[:], a, b, op)
        return o[:]

    # ---------------- pelvis R ----------------
    ms0s1 = mul(s0, s1, "ms01")
    mc0s1 = mul(c0, s1, "mc01")
    P1x = nmul(s0, c1, "P1x")
    P1y = mul(c0, c1, "P1y")
    P1z = s1                                        # alias
    P0x = comb(mul(c0, c2), mul(ms0s1, s2, "m2"), ALU.subtract, "P0x", per)
    P0y = comb(mul(s0, c2), mul(mc0s1, s2, "m2"), ALU.add, "P0y", per)
    P0z = nmul(c1, s2, "P0z")
    P2x = comb(mul(c0, s2), mul(ms0s1, c2, "m2"), ALU.add, "P2x", per)
    P2y = comb(mul(s0, s2), mul(mc0s1, c2, "m2"), ALU.subtract, "P2y", per)
    P2z = mul(c1, c2, "P2z")
    P0 = (P0x, P0y, P0z)
    P1 = (P1x, P1y, P1z)
    P2 = (P2x, P2y, P2z)

    # ---------------- torso R = Rpel @ Rz3 @ Ry4 ----------------
    def colupd(cc, ss, A3, B3, tagp, pool=None, fd=FD):
        """returns cc*A + ss*B per component."""
        out = []
        for i, (a, b) in enumerate(zip(A3, B3)):
            out.append(comb(mul(cc, a, "ca", fd), mul(ss, b, "cb", fd), ALU.add,
                            f"{tagp}{i}", pool, fd))
        return tuple(out)

    def colupd_sub(cc, ss, A3, B3, tagp, pool=None, fd=FD):
        """returns cc*A - ss*B per component."""
        out = []
        for i, (a, b) in enumerate(zip(A3, B3)):
            out.append(comb(mul(cc, a, "ca", fd), mul(ss, b, "cb", fd),
                            ALU.subtract, f"{tagp}{i}", pool, fd))
        return tuple(out)

    D0t = colupd(c3, s3, P0, P1, "D0t")
    D1t = colupd_sub(c3, s3, P1, P0, "D1t", per)       # E1 = D1t
    E0 = colupd_sub(c4, s4, D0t, P2, "E0", per)
    E2 = colupd(s4, c4, D0t, P2, "E2", per)

    # ---------------- phase A translations ----------------
    scH = fresh("scH")
    V.tensor_copy(scH[:], mk(Xa, 25, [[K, FD]]))       # scale as fp16

    TP = [per.tile([128, FDC], f16, tag=f"TP{c}", name=f"TP{c}")
          for c in range(3)]

    def tp_slice(c, i):
        return TP[c][:, i * FD:(i + 1) * FD]

    for c in range(3):
        # torso t = S3*scale*D1 -> Y joint1 + TP[neck]
        V.scalar_tensor_tensor(ycol(30 + c), scH[:], S3, D1t[c],
                               ALU.mult, ALU.mult)
        V.scalar_tensor_tensor(tp_slice(c, 0), scH[:], S3, D1t[c],
                               ALU.mult, ALU.mult)
        # hips: +-S2*scale*P0 -> TP legs; left hip -> Y
        V.scalar_tensor_tensor(tp_slice(c, 1), scH[:], S2, P0[c],
                               ALU.mult, ALU.mult)
        V.scalar_tensor_tensor(tp_slice(c, 2), scH[:], -S2, P0[c],
                               ALU.mult, ALU.mult)
        A.copy(ycol(33 + c), tp_slice(c, 1))
        # shoulders: t_tor +- S8*scale*E0 -> TP arms; left shoulder -> Y
        u = fresh("shu")
        V.scalar_tensor_tensor(u[:], scH[:], S8, E0[c], ALU.mult, ALU.mult)
        tt(tp_slice(c, 3), tp_slice(c, 0), u[:], ALU.add)
        tt(tp_slice(c, 4), tp_slice(c, 0), u[:], ALU.subtract)
        A.copy(ycol(36 + c), tp_slice(c, 3))

    # ---------------- batched parent-R tiles ----------------
    # chains: 0=neck(E), 1,2=legs(P), 3,4=arms(E)
    PR = [[per.tile([128, FDC], f16, tag=f"PR{c}{i}", name=f"PR{c}{i}")
           for i in range(3)] for c in range(3)]
    for ci, (Ecol, Pcol) in enumerate(((E0, P0), (D1t, P1), (E2, P2))):
        for i in range(3):
            dst = PR[ci][i][:]
            e = Ecol[i]
            p = Pcol[i]

            def bc2(src):
                return bass.AP(src.tensor, src.offset,
                               [list(src.ap[0]), [0, 2], [1, FD]])

            A.copy(mk(dst, 0, [[1, FD]]), e)
            A.copy(mk(dst, FD, [[1, 2 * FD]]), bc2(p))
            A.copy(mk(dst, 3 * FD, [[1, 2 * FD]]), bc2(e))

    def prc(c):
        return tuple(PR[c][i][:] for i in range(3))

    cA, sA = (t[:] for t in CS[0])
    cB, sB = (t[:] for t in CS[1])
    cG, sG = (t[:] for t in CS[2])
    cD, sD = (t[:] for t in CS[3])

    # ---------------- batched chain (FD=1280 ops) ----------------
    bD0 = colupd(cA, sA, prc(0), prc(1), "bD0", per, FDC)
    bD1 = colupd_sub(cA, sA, prc(1), prc(0), "bD1", per, FDC)
    bK1 = colupd(cB, sB, bD1, prc(2), "bK1", per, FDC)
    bK2 = colupd_sub(cB, sB, prc(2), bD1, "bK2", per, FDC)
    bK2p = colupd(sG, cG, bD0, bK2, "bD1", per, FDC)  # reuse bD1 slots
    bC1 = colupd(cD, sD, bK1, bK2p, "bD0", per, FDC)  # reuse bD0 slots

    # dT tiles: per-chain signed bone length * scale
    scB = fresh("scB", FDC)
    V.tensor_copy(scB[:], mk(scH[:], 0, [[0, 5], [1, FD]]))
    dT1 = fresh("dT1", FDC)
    dT2 = fresh("dT2", FDC)
    for i in range(5):
        sl = slice(i * FD, (i + 1) * FD)
        A.mul(dT1[:, sl], scB[:, sl], DT1[i])
        A.mul(dT2[:, sl], scB[:, sl], DT2[i])

    for c in range(3):
        u = fresh("btr", FDC)
        tt(u[:], dT1[:], bK1[c], ALU.mult)
        tt(ygrp(c), TP[c][:], u[:], ALU.add)             # knee-level joints
        u2 = fresh("btr2", FDC)
        tt(u2[:], dT2[:], bC1[c], ALU.mult)
        tt(ygrp(15 + c), ygrp(c), u2[:], ALU.add)        # distal joints

    HY = J * FD // 2
    for h in range(2):
        nc.gpsimd.dma_start(bass.AP(y.tensor, base * J + h * HY,
                                    [[FD * J, 128], [1, HY]]),
                            Y[:, h * HY:(h + 1) * HY])


# ---------------------------------------------------------------------------
# Cached PJRT runner: jit(shard_map(bass_exec)) built once; the previous
# call's device output buffers (already copied to host) are donated back as
# the custom-call output operands, so steady-state wire traffic is just
# x (f16 up) + y (f16 down).
# ---------------------------------------------------------------------------
_STATE = None


def _init():
    nc = build()
    b2j.install_neuronx_cc_hook()

    partition_name = (nc.partition_id_tensor.name
                      if nc.partition_id_tensor else None)
    in_names, out_names, out_avals = [], [], []
    for alloc in nc.m.functions[0].allocations:
        if not isinstance(alloc, mybir.MemoryLocationSet):
            continue
        name = alloc.memorylocations[0].name
        if alloc.kind == "ExternalInput":
            if name != partition_name:
                in_names.append(name)
        elif alloc.kind == "ExternalOutput":
            out_names.append(name)
            out_avals.append(jax.core.ShapedArray(
                tuple(alloc.tensor_shape), mybir.dt.np(alloc.dtype)))
    assert in_names == ["x"] and out_names == ["y"], (in_names, out_names)
    n_params = len(in_names)
    in_names_all = in_names + out_names
    if partition_name is not None:
        in_names_all.append(partition_name)
    donate = tuple(range(n_params, n_params + len(out_names)))

    def _body(*args):
        operands = list(args)
        if partition_name is not None:
            operands.append(b2j.partition_id_tensor())
        outs = b2j._bass_exec_p.bind(
            *operands,
            out_avals=tuple(out_avals),
            in_names=tuple(in_names_all),
            out_names=tuple(out_names),
            lowering_input_output_aliases=(),
            sim_require_finite=True,
            sim_require_nnan=True,
            nc=nc,
        )
        return tuple(outs)

    devices = jax.devices()[:NCORE]
    assert len(devices) == NCORE
    mesh = Mesh(np.asarray(devices), ("core",))
    nin = n_params + len(out_names)
    fn = jax.jit(
        shard_map(_body, mesh=mesh,
                  in_specs=(PartitionSpec("core"),) * nin,
                  out_specs=(PartitionSpec("core"),) * len(out_names),
                  check_rep=False),
        donate_argnums=donate,
        keep_unused=True,
    )
    return {"fn": fn, "prev": None}


# final-output assembly map: shipped block i -> joint j
_JMAP = [(0, 2), (1, 5), (2, 8), (3, 11), (4, 14),      # knee-level
         (5, 3), (6, 6), (7, 9), (8, 12), (9, 15),      # distal
         (10, 1), (11, 4), (12, 10)]                     # torso, lhip, lsh


def kernel(x: np.ndarray) -> np.ndarray:
    global _STATE
    if _STATE is None:
        _STATE = _init()
    st = _STATE

    x16 = np.asarray(x, dtype=np.float16)
    prev = st["prev"]
    if prev is None:
        prev = np.zeros((N, J), np.float16)
    out, = st["fn"](x16, prev)
    y16 = np.asarray(out)
    st["prev"] = out                     # donate next call (already fetched)

    B = y16.reshape(N, 13, 3).astype(np.float32)
    res = np.empty((N, 51), np.float32)
    res[:, 0:3] = 0.0                                   # pelvis
    for i, j in _JMAP:
        res[:, 3 * j:3 * j + 3] = B[:, i, :]
    res[:, 21:24] = -res[:, 12:15]                      # rhip = -lhip
    res[:, 39:42] = 2.0 * res[:, 3:6] - res[:, 30:33]   # rsh = 2*torso - lsh
    res[:, 48:51] = 0.5 * (res[:, 18:21] + res[:, 24:27])  # thorax
    return res


# revision 8
# speedup vs baseline: 4.3431x; 1.6461x over previous
"""Trainium2 Bass kernel for nn_KinematicLayer: batched forward kinematics.

Full inputs x:[524288,26] f32 -> out:[524288,51] f32.

End-to-end wall time is dominated by the axon host<->device tunnel
(~50-80 MB/s, serial), so the kernel minimizes wire bytes:
  - x is uploaded as f16 ([N,26], 27 MB instead of 55 MB);
  - the device returns only the 13 non-derivable joint positions as f16
    ([N,39], 41 MB instead of 107 MB).  Host derives the rest:
    p0 = 0, p7 = -p4, p13 = 2*p1 - p10, thorax = (p6+p8)/2.
  - the jitted executable is cached across calls, and the previous call's
    (already fetched) device output buffers are donated back as the
    custom-call output operands, so no zero-buffer upload per call.

Device compute (per core: 65536 samples, 2 chunks of 128x256): per-sample
state tracked as (R 3x3, t 3); the five limb chains share one instruction
stream batched along the free dim (FD=1280).  Trig via half-angle
identities keeps every ACT Sin argument inside the spline's valid
[-pi,pi] range: u=sin(x/2), w=sin(x/4), v=1-2w^2=cos(x/2), cos=1-2u^2,
sin=2uv.  Intermediates fp16 (DVE 2x mode).
"""
import numpy as np
import jax
from jax.sharding import Mesh, PartitionSpec
from jax.experimental.shard_map import shard_map

import concourse.bass as bass
import concourse.tile as tile
import concourse.bass2jax as b2j
from concourse import bacc, mybir

AF = mybir.ActivationFunctionType
ALU = mybir.AluOpType
f32, f16 = mybir.dt.float32, mybir.dt.float16

N, K = 524288, 26
J = 39                      # 13 joints x 3 shipped to host
NCORE = 8
NPC = N // NCORE            # 65536 samples per core
FD = 256                    # samples per partition per chunk
CHUNK = 128 * FD            # 32768 samples per chunk
NCHUNK = NPC // CHUNK       # 2

_S = np.array([300.0, 350.0, 75.0, 400.0, 73.96, 249.03, 250.0, 250.0, 170.0],
              np.float32) / 300.0
S0, S1, S2, S3, S4, S5, S6, S7, S8 = [float(v) for v in _S]

# chain order: (neck, Lleg, Rleg, Larm, Rarm); euler angle bases 5,9,13,17,21
DT1 = [S4, -S1, -S1, -S7, -S7]   # signed first-translation lengths
DT2 = [S5, -S0, -S0, -S6, -S6]   # signed distal-translation lengths

FDC = 5 * FD                 # batched chain free dim

# Output column layout (all f16): the 13 shipped joints in final joint
# order [1,2,3,4,5,6,8,9,10,11,12,14,15], 3 cols each — so host assembly
# is a handful of wide contiguous block copies.
#   cols  0: 3 j1 torso | 3: 6 j2 | 6: 9 j3 | 9:12 j4 | 12:15 j5 | 15:18 j6
#   cols 18:21 j8 | 21:24 j9 | 24:27 j10 | 27:30 j11 | 30:33 j12
#   cols 33:36 j14 | 36:39 j15
# Knee-level joints (2,5,8,11,14) land at col bases (3,12,18,27,33):
# chains {0,2,4} -> 3,18,33 (stride 15), chains {1,3} -> 12,27 (stride 15).
# Distal joints (3,6,9,12,15) at (6,15,21,30,36): same two-group split.


def mk(ap, off, dims):
    """Custom free-dim AP on the same tile/tensor (keeps partition dim)."""
    return bass.AP(ap.tensor, ap.offset + off, [list(ap.ap[0])] + dims)


def build():
    nc = bacc.Bacc("TRN2", target_bir_lowering=False, debug=False,
                   num_devices=NCORE)
    x = nc.dram_tensor("x", [NPC, K], f16, kind="ExternalInput").ap()
    y = nc.dram_tensor("y", [NPC, J], f16, kind="ExternalOutput").ap()

    with tile.TileContext(nc) as tc:
        with (
            tc.tile_pool(name="io", bufs=1) as io,
            tc.tile_pool(name="per", bufs=1) as per,
            tc.tile_pool(name="scr", bufs=1) as scr,
        ):
            for ch in range(NCHUNK):
                build_chunk(nc, tc, io, per, scr, x, y, ch)
    nc.compile()
    return nc


def build_chunk(nc, tc, io, per, scr, x, y, ch):
    V, A = nc.vector, nc.scalar
    base = ch * CHUNK

    X = io.tile([128, K * FD], f16, tag="X")
    HX = K * FD // 2
    for h in range(2):
        nc.gpsimd.dma_start(X[:, h * HX:(h + 1) * HX],
                            bass.AP(x.tensor, base * K + h * HX,
                                    [[FD * K, 128], [1, HX]]))
    Y = io.tile([128, J * FD], f16, tag="Y")
    Xa = X[:]
    Ya = Y[:]

    def ycol(c):                       # output scalar col c strided [128,FD]
        return mk(Ya, c, [[J, FD]])

    def ygrpA(c0):                     # chains 0,2,4 -> 3 joints stride 15
        return mk(Ya, c0, [[15, 3], [J, FD]])

    def ygrpB(c0):                     # chains 1,3 -> 2 joints stride 15
        return mk(Ya, c0, [[15, 2], [J, FD]])

    def srcA(t):                       # chain-major [128,5*FD] -> chains 0,2,4
        a = t if isinstance(t, bass.AP) else t[:]
        return bass.AP(a.tensor, a.offset, [list(a.ap[0]), [2 * FD, 3], [1, FD]])

    def srcB(t):                       # chains 1,3
        a = t if isinstance(t, bass.AP) else t[:]
        return bass.AP(a.tensor, a.offset + FD,
                       [list(a.ap[0]), [2 * FD, 2], [1, FD]])

    # ---------------- trig: 5 groups ----------------
    def trig(tag, xap, n):
        fd = n * FD
        u = scr.tile([128, fd], f16, tag="trigU", name="trigU")
        w = scr.tile([128, fd], f16, tag="trigW", name="trigW")
        A.activation(u[:], xap, AF.Sin, scale=0.5)
        A.activation(w[:], xap, AF.Sin, scale=0.25)
        q = scr.tile([128, fd], f16, tag="trigQ", name="trigQ")
        c = per.tile([128, fd], f16, tag=f"C{tag}", name=f"C{tag}")
        s = per.tile([128, fd], f16, tag=f"S{tag}", name=f"S{tag}")
        A.square(q[:], u[:])
        V.tensor_scalar(c[:], q[:], -2.0, 1.0, ALU.mult, ALU.add)
        A.square(q[:], w[:])
        V.tensor_scalar(q[:], q[:], -2.0, 1.0, ALU.mult, ALU.add)  # v in q
        V.scalar_tensor_tensor(s[:], u[:], 2.0, q[:], ALU.mult, ALU.mult)
        return c, s

    Cpt, Spt = trig("pt", mk(Xa, 0, [[1, 5], [K, FD]]), 5)
    CS = [trig(f"p{j}", mk(Xa, 5 + j, [[4, 5], [K, FD]]), 5) for j in range(4)]

    def pt(t, i):
        return t[:, i * FD:(i + 1) * FD]

    c0, s0 = pt(Cpt, 0), pt(Spt, 0)
    c1, s1 = pt(Cpt, 1), pt(Spt, 1)
    c2, s2 = pt(Cpt, 2), pt(Spt, 2)
    c3, s3 = pt(Cpt, 3), pt(Spt, 3)
    c4, s4 = pt(Cpt, 4), pt(Spt, 4)

    def tt(out, a, b, op):
        V.tensor_tensor(out, a, b, op)

    def fresh(tag, fd=FD, dt=f16, pool=None):
        return (pool or scr).tile([128, fd], dt, tag=tag, name=tag)

    def mul(a, b, tag="m", fd=FD):
        o = fresh(tag, fd=fd)
        tt(o[:], a, b, ALU.mult)
        return o[:]

    def nmul(a, b, tag="m"):           # -(a*b)
        o = fresh(tag)
        V.scalar_tensor_tensor(o[:], a, -1.0, b, ALU.mult, ALU.mult)
        return o[:]

    def comb(a, b, op, tag="m", pool=None, fd=FD):
        o = fresh(tag, fd=fd, pool=pool)
        tt(o[:], a, b, op)
        return o[:]

    # ---------------- pelvis R ----------------
    ms0s1 = mul(s0, s1, "ms01")
    mc0s1 = mul(c0, s1, "mc01")
    P1x = nmul(s0, c1, "P1x")
    P1y = mul(c0, c1, "P1y")
    P1z = s1                                        # alias
    P0x = comb(mul(c0, c2), mul(ms0s1, s2, "m2"), ALU.subtract, "P0x", per)
    P0y = comb(mul(s0, c2), mul(mc0s1, s2, "m2"), ALU.add, "P0y", per)
    P0z = nmul(c1, s2, "P0z")
    P2x = comb(mul(c0, s2), mul(ms0s1, c2, "m2"), ALU.add, "P2x", per)
    P2y = comb(mul(s0, s2), mul(mc0s1, c2, "m2"), ALU.subtract, "P2y", per)
    P2z = mul(c1, c2, "P2z")
    P0 = (P0x, P0y, P0z)
    P1 = (P1x, P1y, P1z)
    P2 = (P2x, P2y, P2z)

    # ---------------- torso R = Rpel @ Rz3 @ Ry4 ----------------
    def colupd(cc, ss, A3, B3, tagp, pool=None, fd=FD):
        """returns cc*A + ss*B per component."""
        out = []
        for i, (a, b) in enumerate(zip(A3, B3)):
            out.append(comb(mul(cc, a, "ca", fd), mul(ss, b, "cb", fd), ALU.add,
                            f"{tagp}{i}", pool, fd))
        return tuple(out)

    def colupd_sub(cc, ss, A3, B3, tagp, pool=None, fd=FD):
        """returns cc*A - ss*B per component."""
        out = []
        for i, (a, b) in enumerate(zip(A3, B3)):
            out.append(comb(mul(cc, a, "ca", fd), mul(ss, b, "cb", fd),
                            ALU.subtract, f"{tagp}{i}", pool, fd))
        return tuple(out)

    D0t = colupd(c3, s3, P0, P1, "D0t")
    D1t = colupd_sub(c3, s3, P1, P0, "D1t", per)       # E1 = D1t
    E0 = colupd_sub(c4, s4, D0t, P2, "E0", per)
    E2 = colupd(s4, c4, D0t, P2, "E2", per)

    # ---------------- phase A translations ----------------
    scH = fresh("scH")
    V.tensor_copy(scH[:], mk(Xa, 25, [[K, FD]]))       # scale as fp16

    TP = [per.tile([128, FDC], f16, tag=f"TP{c}", name=f"TP{c}")
          for c in range(3)]

    def tp_slice(c, i):
        return TP[c][:, i * FD:(i + 1) * FD]

    for c in range(3):
        # torso t = S3*scale*D1 -> Y joint1 + TP[neck]
        V.scalar_tensor_tensor(ycol(0 + c), scH[:], S3, D1t[c],
                               ALU.mult, ALU.mult)
        V.scalar_tensor_tensor(tp_slice(c, 0), scH[:], S3, D1t[c],
                               ALU.mult, ALU.mult)
        # hips: +-S2*scale*P0 -> TP legs; left hip -> Y
        V.scalar_tensor_tensor(tp_slice(c, 1), scH[:], S2, P0[c],
                               ALU.mult, ALU.mult)
        V.scalar_tensor_tensor(tp_slice(c, 2), scH[:], -S2, P0[c],
                               ALU.mult, ALU.mult)
        A.copy(ycol(9 + c), tp_slice(c, 1))
        # shoulders: t_tor +- S8*scale*E0 -> TP arms; left shoulder -> Y
        u = fresh("shu")
        V.scalar_tensor_tensor(u[:], scH[:], S8, E0[c], ALU.mult, ALU.mult)
        tt(tp_slice(c, 3), tp_slice(c, 0), u[:], ALU.add)
        tt(tp_slice(c, 4), tp_slice(c, 0), u[:], ALU.subtract)
        A.copy(ycol(24 + c), tp_slice(c, 3))

    # ---------------- batched parent-R tiles ----------------
    # chains: 0=neck(E), 1,2=legs(P), 3,4=arms(E)
    PR = [[per.tile([128, FDC], f16, tag=f"PR{c}{i}", name=f"PR{c}{i}")
           for i in range(3)] for c in range(3)]
    for ci, (Ecol, Pcol) in enumerate(((E0, P0), (D1t, P1), (E2, P2))):
        for i in range(3):
            dst = PR[ci][i][:]
            e = Ecol[i]
            p = Pcol[i]

            def bc2(src):
                return bass.AP(src.tensor, src.offset,
                               [list(src.ap[0]), [0, 2], [1, FD]])

            A.copy(mk(dst, 0, [[1, FD]]), e)
            A.copy(mk(dst, FD, [[1, 2 * FD]]), bc2(p))
            A.copy(mk(dst, 3 * FD, [[1, 2 * FD]]), bc2(e))

    def prc(c):
        return tuple(PR[c][i][:] for i in range(3))

    cA, sA = (t[:] for t in CS[0])
    cB, sB = (t[:] for t in CS[1])
    cG, sG = (t[:] for t in CS[2])
    cD, sD = (t[:] for t in CS[3])

    # ---------------- batched chain (FD=1280 ops) ----------------
    bD0 = colupd(cA, sA, prc(0), prc(1), "bD0", per, FDC)
    bD1 = colupd_sub(cA, sA, prc(1), prc(0), "bD1", per, FDC)
    bK1 = colupd(cB, sB, bD1, prc(2), "bK1", per, FDC)
    bK2 = colupd_sub(cB, sB, prc(2), bD1, "bK2", per, FDC)
    bK2p = colupd(sG, cG, bD0, bK2, "bD1", per, FDC)  # reuse bD1 slots
    bC1 = colupd(cD, sD, bK1, bK2p, "bD0", per, FDC)  # reuse bD0 slots

    # dT tiles: per-chain signed bone length * scale
    scB = fresh("scB", FDC)
    V.tensor_copy(scB[:], mk(scH[:], 0, [[0, 5], [1, FD]]))
    dT1 = fresh("dT1", FDC)
    dT2 = fresh("dT2", FDC)
    for i in range(5):
        sl = slice(i * FD, (i + 1) * FD)
        A.mul(dT1[:, sl], scB[:, sl], DT1[i])
        A.mul(dT2[:, sl], scB[:, sl], DT2[i])

    for c in range(3):
        u = fresh("btr", FDC)
        tt(u[:], dT1[:], bK1[c], ALU.mult)
        kn = fresh("kn", FDC)
        tt(kn[:], TP[c][:], u[:], ALU.add)               # knee-level joints
        u2 = fresh("btr2", FDC)
        tt(u2[:], dT2[:], bC1[c], ALU.mult)
        ds = fresh("ds", FDC)
        tt(ds[:], kn[:], u2[:], ALU.add)                 # distal joints
        A.copy(ygrpA(3 + c), srcA(kn))
        A.copy(ygrpB(12 + c), srcB(kn))
        A.copy(ygrpA(6 + c), srcA(ds))
        A.copy(ygrpB(15 + c), srcB(ds))

    HY = J * FD // 2
    for h in range(2):
        nc.gpsimd.dma_start(bass.AP(y.tensor, base * J + h * HY,
                                    [[FD * J, 128], [1, HY]]),
                            Y[:, h * HY:(h + 1) * HY])


# ---------------------------------------------------------------------------
# Cached PJRT runner: jit(shard_map(bass_exec)) built once; the previous
# call's device output buffers (already copied to host) are donated back as
# the custom-call output operands, so steady-state wire traffic is just
# x (f16 up) + y (f16 down).
# ---------------------------------------------------------------------------
_STATE = None


def _init():
    nc = build()
    b2j.install_neuronx_cc_hook()

    partition_name = (nc.partition_id_tensor.name
                      if nc.partition_id_tensor else None)
    in_names, out_names, out_avals = [], [], []
    for alloc in nc.m.functions[0].allocations:
        if not isinstance(alloc, mybir.MemoryLocationSet):
            continue
        name = alloc.memorylocations[0].name
        if alloc.kind == "ExternalInput":
            if name != partition_name:
                in_names.append(name)
        elif alloc.kind == "ExternalOutput":
            out_names.append(name)
            out_avals.append(jax.core.ShapedArray(
                tuple(alloc.tensor_shape), mybir.dt.np(alloc.dtype)))
    assert in_names == ["x"] and out_names == ["y"], (in_names, out_names)
    n_params = len(in_names)
    in_names_all = in_names + out_names
    if partition_name is not None:
        in_names_all.append(partition_name)
    donate = tuple(range(n_params, n_params + len(out_names)))

    def _body(*args):
        operands = list(args)
        if partition_name is not None:
            operands.append(b2j.partition_id_tensor())
        outs = b2j._bass_exec_p.bind(
            *operands,
            out_avals=tuple(out_avals),
            in_names=tuple(in_names_all),
            out_names=tuple(out_names),
            lowering_input_output_aliases=(),
            sim_require_finite=True,
            sim_require_nnan=True,
            nc=nc,
        )
        return tuple(outs)

    devices = jax.devices()[:NCORE]
    assert len(devices) == NCORE
    mesh = Mesh(np.asarray(devices), ("core",))
    nin = n_params + len(out_names)
    fn = jax.jit(
        shard_map(_body, mesh=mesh,
                  in_specs=(PartitionSpec("core"),) * nin,
                  out_specs=(PartitionSpec("core"),) * len(out_names),
                  check_rep=False),
        donate_argnums=donate,
        keep_unused=True,
    )
    return {"fn": fn, "prev": None}


def _assemble(res, y16):
    """Expand shipped [*,39] f16 block into final [*,51] f32 rows."""
    B = y16.astype(np.float32)
    res[:, 0:3] = 0.0                                   # pelvis
    res[:, 3:21] = B[:, 0:18]                           # j1..j6
    res[:, 24:30] = B[:, 18:24]                         # j8, j9
    res[:, 30:39] = B[:, 24:33]                         # j10, j11, j12
    res[:, 42:48] = B[:, 33:39]                         # j14, j15
    res[:, 21:24] = -B[:, 9:12]                         # rhip = -lhip
    res[:, 39:42] = 2.0 * B[:, 0:3] - B[:, 24:27]       # rsh = 2*torso - lsh
    res[:, 48:51] = 0.5 * (B[:, 15:18] + B[:, 18:21])   # thorax = (j6+j8)/2


def kernel(x: np.ndarray) -> np.ndarray:
    global _STATE
    if _STATE is None:
        _STATE = _init()
    st = _STATE

    x16 = np.asarray(x, dtype=np.float16)
    prev = st["prev"]
    if prev is None:
        prev = np.zeros((N, J), np.float16)
    out, = st["fn"](x16, prev)

    # Stream shards: queue all device->host copies, then assemble each
    # shard's rows while later shards are still on the wire.
    shards = sorted(out.addressable_shards, key=lambda s: s.index[0].start or 0)
    datas = [s.data for s in shards]
    for d in datas:
        try:
            d.copy_to_host_async()
        except Exception:
            pass
    res = np.empty((N, 51), np.float32)
    r0 = 0
    for d in datas:
        y16 = np.asarray(d)
        r1 = r0 + y16.shape[0]
        _assemble(res[r0:r1], y16)
        r0 = r1
    assert r0 == N
    st["prev"] = out                     # donate next call (already fetched)
    return res


# revision 10
# speedup vs baseline: 4.8269x; 1.1114x over previous
"""Trainium2 Bass kernel for nn_KinematicLayer: batched forward kinematics.

Full inputs x:[524288,26] f32 -> out:[524288,51] f32.

End-to-end wall time is dominated by the axon host<->device tunnel
(~50-80 MB/s, serial), so the kernel minimizes wire bytes:
  - x is uploaded as f16 ([N,26], 27 MB instead of 55 MB);
  - the device returns only the 13 non-derivable joint positions as f16
    ([N,39], 41 MB instead of 107 MB).  Host derives the rest:
    p0 = 0, p7 = -p4, p13 = 2*p1 - p10, thorax = (p6+p8)/2.
  - the jitted executable is cached across calls, and the previous call's
    (already fetched) device output buffers are donated back as the
    custom-call output operands, so no zero-buffer upload per call.

Device compute (per core: 65536 samples, 2 chunks of 128x256): per-sample
state tracked as (R 3x3, t 3); the five limb chains share one instruction
stream batched along the free dim (FD=1280).  Trig via half-angle
identities keeps every ACT Sin argument inside the spline's valid
[-pi,pi] range: u=sin(x/2), w=sin(x/4), v=1-2w^2=cos(x/2), cos=1-2u^2,
sin=2uv.  Intermediates fp16 (DVE 2x mode).
"""
import numpy as np
import jax
from jax.sharding import Mesh, PartitionSpec
from jax.experimental.shard_map import shard_map

import concourse.bass as bass
import concourse.tile as tile
import concourse.bass2jax as b2j
from concourse import bacc, mybir

AF = mybir.ActivationFunctionType
ALU = mybir.AluOpType
f32, f16 = mybir.dt.float32, mybir.dt.float16

N, K = 524288, 26
J = 39                      # 13 joints x 3 shipped to host
NCORE = 8
NGRP = 2                    # pipeline groups (upload/exec/download overlap)
NG = N // NGRP              # 262144 samples per group
NPC = NG // NCORE           # 32768 samples per core per group
FD = 256                    # samples per partition per chunk
CHUNK = 128 * FD            # 32768 samples per chunk
NCHUNK = NPC // CHUNK       # 1

_S = np.array([300.0, 350.0, 75.0, 400.0, 73.96, 249.03, 250.0, 250.0, 170.0],
              np.float32) / 300.0
S0, S1, S2, S3, S4, S5, S6, S7, S8 = [float(v) for v in _S]

# chain order: (neck, Lleg, Rleg, Larm, Rarm); euler angle bases 5,9,13,17,21
DT1 = [S4, -S1, -S1, -S7, -S7]   # signed first-translation lengths
DT2 = [S5, -S0, -S0, -S6, -S6]   # signed distal-translation lengths

FDC = 5 * FD                 # batched chain free dim

# Output column layout (all f16): the 13 shipped joints in final joint
# order [1,2,3,4,5,6,8,9,10,11,12,14,15], 3 cols each — so host assembly
# is a handful of wide contiguous block copies.
#   cols  0: 3 j1 torso | 3: 6 j2 | 6: 9 j3 | 9:12 j4 | 12:15 j5 | 15:18 j6
#   cols 18:21 j8 | 21:24 j9 | 24:27 j10 | 27:30 j11 | 30:33 j12
#   cols 33:36 j14 | 36:39 j15
# Knee-level joints (2,5,8,11,14) land at col bases (3,12,18,27,33):
# chains {0,2,4} -> 3,18,33 (stride 15), chains {1,3} -> 12,27 (stride 15).
# Distal joints (3,6,9,12,15) at (6,15,21,30,36): same two-group split.


def mk(ap, off, dims):
    """Custom free-dim AP on the same tile/tensor (keeps partition dim)."""
    return bass.AP(ap.tensor, ap.offset + off, [list(ap.ap[0])] + dims)


def build():
    nc = bacc.Bacc("TRN2", target_bir_lowering=False, debug=False,
                   num_devices=NCORE)
    x = nc.dram_tensor("x", [NPC, K], f16, kind="ExternalInput").ap()
    y = nc.dram_tensor("y", [NPC, J], f16, kind="ExternalOutput").ap()

    with tile.TileContext(nc) as tc:
        with (
            tc.tile_pool(name="io", bufs=1) as io,
            tc.tile_pool(name="per", bufs=1) as per,
            tc.tile_pool(name="scr", bufs=1) as scr,
        ):
            for ch in range(NCHUNK):
                build_chunk(nc, tc, io, per, scr, x, y, ch)
    nc.compile()
    return nc


def build_chunk(nc, tc, io, per, scr, x, y, ch):
    V, A = nc.vector, nc.scalar
    base = ch * CHUNK

    X = io.tile([128, K * FD], f16, tag="X")
    HX = K * FD // 2
    for h in range(2):
        nc.gpsimd.dma_start(X[:, h * HX:(h + 1) * HX],
                            bass.AP(x.tensor, base * K + h * HX,
                                    [[FD * K, 128], [1, HX]]))
    Y = io.tile([128, J * FD], f16, tag="Y")
    Xa = X[:]
    Ya = Y[:]

    def ycol(c):                       # output scalar col c strided [128,FD]
        return mk(Ya, c, [[J, FD]])

    def ygrpA(c0):                     # chains 0,2,4 -> 3 joints stride 15
        return mk(Ya, c0, [[15, 3], [J, FD]])

    def ygrpB(c0):                     # chains 1,3 -> 2 joints stride 15
        return mk(Ya, c0, [[15, 2], [J, FD]])

    def srcA(t):                       # chain-major [128,5*FD] -> chains 0,2,4
        a = t if isinstance(t, bass.AP) else t[:]
        return bass.AP(a.tensor, a.offset, [list(a.ap[0]), [2 * FD, 3], [1, FD]])

    def srcB(t):                       # chains 1,3
        a = t if isinstance(t, bass.AP) else t[:]
        return bass.AP(a.tensor, a.offset + FD,
                       [list(a.ap[0]), [2 * FD, 2], [1, FD]])

    # ---------------- trig: 5 groups ----------------
    def trig(tag, xap, n):
        fd = n * FD
        u = scr.tile([128, fd], f16, tag="trigU", name="trigU")
        w = scr.tile([128, fd], f16, tag="trigW", name="trigW")
        A.activation(u[:], xap, AF.Sin, scale=0.5)
        A.activation(w[:], xap, AF.Sin, scale=0.25)
        q = scr.tile([128, fd], f16, tag="trigQ", name="trigQ")
        c = per.tile([128, fd], f16, tag=f"C{tag}", name=f"C{tag}")
        s = per.tile([128, fd], f16, tag=f"S{tag}", name=f"S{tag}")
        A.square(q[:], u[:])
        V.tensor_scalar(c[:], q[:], -2.0, 1.0, ALU.mult, ALU.add)
        A.square(q[:], w[:])
        V.tensor_scalar(q[:], q[:], -2.0, 1.0, ALU.mult, ALU.add)  # v in q
        V.scalar_tensor_tensor(s[:], u[:], 2.0, q[:], ALU.mult, ALU.mult)
        return c, s

    Cpt, Spt = trig("pt", mk(Xa, 0, [[1, 5], [K, FD]]), 5)
    CS = [trig(f"p{j}", mk(Xa, 5 + j, [[4, 5], [K, FD]]), 5) for j in range(4)]

    def pt(t, i):
        return t[:, i * FD:(i + 1) * FD]

    c0, s0 = pt(Cpt, 0), pt(Spt, 0)
    c1, s1 = pt(Cpt, 1), pt(Spt, 1)
    c2, s2 = pt(Cpt, 2), pt(Spt, 2)
    c3, s3 = pt(Cpt, 3), pt(Spt, 3)
    c4, s4 = pt(Cpt, 4), pt(Spt, 4)

    def tt(out, a, b, op):
        V.tensor_tensor(out, a, b, op)

    def fresh(tag, fd=FD, dt=f16, pool=None):
        return (pool or scr).tile([128, fd], dt, tag=tag, name=tag)

    def mul(a, b, tag="m", fd=FD):
        o = fresh(tag, fd=fd)
        tt(o[:], a, b, ALU.mult)
        return o[:]

    def nmul(a, b, tag="m"):           # -(a*b)
        o = fresh(tag)
        V.scalar_tensor_tensor(o[:], a, -1.0, b, ALU.mult, ALU.mult)
        return o[:]

    def comb(a, b, op, tag="m", pool=None, fd=FD):
        o = fresh(tag, fd=fd, pool=pool)
        tt(o[:], a, b, op)
        return o[:]

    # ---------------- pelvis R ----------------
    ms0s1 = mul(s0, s1, "ms01")
    mc0s1 = mul(c0, s1, "mc01")
    P1x = nmul(s0, c1, "P1x")
    P1y = mul(c0, c1, "P1y")
    P1z = s1                                        # alias
    P0x = comb(mul(c0, c2), mul(ms0s1, s2, "m2"), ALU.subtract, "P0x", per)
    P0y = comb(mul(s0, c2), mul(mc0s1, s2, "m2"), ALU.add, "P0y", per)
    P0z = nmul(c1, s2, "P0z")
    P2x = comb(mul(c0, s2), mul(ms0s1, c2, "m2"), ALU.add, "P2x", per)
    P2y = comb(mul(s0, s2), mul(mc0s1, c2, "m2"), ALU.subtract, "P2y", per)
    P2z = mul(c1, c2, "P2z")
    P0 = (P0x, P0y, P0z)
    P1 = (P1x, P1y, P1z)
    P2 = (P2x, P2y, P2z)

    # ---------------- torso R = Rpel @ Rz3 @ Ry4 ----------------
    def colupd(cc, ss, A3, B3, tagp, pool=None, fd=FD):
        """returns cc*A + ss*B per component."""
        out = []
        for i, (a, b) in enumerate(zip(A3, B3)):
            out.append(comb(mul(cc, a, "ca", fd), mul(ss, b, "cb", fd), ALU.add,
                            f"{tagp}{i}", pool, fd))
        return tuple(out)

    def colupd_sub(cc, ss, A3, B3, tagp, pool=None, fd=FD):
        """returns cc*A - ss*B per component."""
        out = []
        for i, (a, b) in enumerate(zip(A3, B3)):
            out.append(comb(mul(cc, a, "ca", fd), mul(ss, b, "cb", fd),
                            ALU.subtract, f"{tagp}{i}", pool, fd))
        return tuple(out)

    D0t = colupd(c3, s3, P0, P1, "D0t")
    D1t = colupd_sub(c3, s3, P1, P0, "D1t", per)       # E1 = D1t
    E0 = colupd_sub(c4, s4, D0t, P2, "E0", per)
    E2 = colupd(s4, c4, D0t, P2, "E2", per)

    # ---------------- phase A translations ----------------
    scH = fresh("scH")
    V.tensor_copy(scH[:], mk(Xa, 25, [[K, FD]]))       # scale as fp16

    TP = [per.tile([128, FDC], f16, tag=f"TP{c}", name=f"TP{c}")
          for c in range(3)]

    def tp_slice(c, i):
        return TP[c][:, i * FD:(i + 1) * FD]

    for c in range(3):
        # torso t = S3*scale*D1 -> Y joint1 + TP[neck]
        V.scalar_tensor_tensor(ycol(0 + c), scH[:], S3, D1t[c],
                               ALU.mult, ALU.mult)
        V.scalar_tensor_tensor(tp_slice(c, 0), scH[:], S3, D1t[c],
                               ALU.mult, ALU.mult)
        # hips: +-S2*scale*P0 -> TP legs; left hip -> Y
        V.scalar_tensor_tensor(tp_slice(c, 1), scH[:], S2, P0[c],
                               ALU.mult, ALU.mult)
        V.scalar_tensor_tensor(tp_slice(c, 2), scH[:], -S2, P0[c],
                               ALU.mult, ALU.mult)
        A.copy(ycol(9 + c), tp_slice(c, 1))
        # shoulders: t_tor +- S8*scale*E0 -> TP arms; left shoulder -> Y
        u = fresh("shu")
        V.scalar_tensor_tensor(u[:], scH[:], S8, E0[c], ALU.mult, ALU.mult)
        tt(tp_slice(c, 3), tp_slice(c, 0), u[:], ALU.add)
        tt(tp_slice(c, 4), tp_slice(c, 0), u[:], ALU.subtract)
        A.copy(ycol(24 + c), tp_slice(c, 3))

    # ---------------- batched parent-R tiles ----------------
    # chains: 0=neck(E), 1,2=legs(P), 3,4=arms(E)
    PR = [[per.tile([128, FDC], f16, tag=f"PR{c}{i}", name=f"PR{c}{i}")
           for i in range(3)] for c in range(3)]
    for ci, (Ecol, Pcol) in enumerate(((E0, P0), (D1t, P1), (E2, P2))):
        for i in range(3):
            dst = PR[ci][i][:]
            e = Ecol[i]
            p = Pcol[i]

            def bc2(src):
                return bass.AP(src.tensor, src.offset,
                               [list(src.ap[0]), [0, 2], [1, FD]])

            A.copy(mk(dst, 0, [[1, FD]]), e)
            A.copy(mk(dst, FD, [[1, 2 * FD]]), bc2(p))
            A.copy(mk(dst, 3 * FD, [[1, 2 * FD]]), bc2(e))

    def prc(c):
        return tuple(PR[c][i][:] for i in range(3))

    cA, sA = (t[:] for t in CS[0])
    cB, sB = (t[:] for t in CS[1])
    cG, sG = (t[:] for t in CS[2])
    cD, sD = (t[:] for t in CS[3])

    # ---------------- batched chain (FD=1280 ops) ----------------
    bD0 = colupd(cA, sA, prc(0), prc(1), "bD0", per, FDC)
    bD1 = colupd_sub(cA, sA, prc(1), prc(0), "bD1", per, FDC)
    bK1 = colupd(cB, sB, bD1, prc(2), "bK1", per, FDC)
    bK2 = colupd_sub(cB, sB, prc(2), bD1, "bK2", per, FDC)
    bK2p = colupd(sG, cG, bD0, bK2, "bD1", per, FDC)  # reuse bD1 slots
    bC1 = colupd(cD, sD, bK1, bK2p, "bD0", per, FDC)  # reuse bD0 slots

    # dT tiles: per-chain signed bone length * scale
    scB = fresh("scB", FDC)
    V.tensor_copy(scB[:], mk(scH[:], 0, [[0, 5], [1, FD]]))
    dT1 = fresh("dT1", FDC)
    dT2 = fresh("dT2", FDC)
    for i in range(5):
        sl = slice(i * FD, (i + 1) * FD)
        A.mul(dT1[:, sl], scB[:, sl], DT1[i])
        A.mul(dT2[:, sl], scB[:, sl], DT2[i])

    for c in range(3):
        u = fresh("btr", FDC)
        tt(u[:], dT1[:], bK1[c], ALU.mult)
        kn = fresh("kn", FDC)
        tt(kn[:], TP[c][:], u[:], ALU.add)               # knee-level joints
        u2 = fresh("btr2", FDC)
        tt(u2[:], dT2[:], bC1[c], ALU.mult)
        ds = fresh("ds", FDC)
        tt(ds[:], kn[:], u2[:], ALU.add)                 # distal joints
        A.copy(ygrpA(3 + c), srcA(kn))
        A.copy(ygrpB(12 + c), srcB(kn))
        A.copy(ygrpA(6 + c), srcA(ds))
        A.copy(ygrpB(15 + c), srcB(ds))

    HY = J * FD // 2
    for h in range(2):
        nc.gpsimd.dma_start(bass.AP(y.tensor, base * J + h * HY,
                                    [[FD * J, 128], [1, HY]]),
                            Y[:, h * HY:(h + 1) * HY])


# ---------------------------------------------------------------------------
# Cached PJRT runner: jit(shard_map(bass_exec)) built once; the previous
# call's device output buffers (already copied to host) are donated back as
# the custom-call output operands, so steady-state wire traffic is just
# x (f16 up) + y (f16 down).
# ---------------------------------------------------------------------------
_STATE = None


def _init():
    nc = build()
    b2j.install_neuronx_cc_hook()

    partition_name = (nc.partition_id_tensor.name
                      if nc.partition_id_tensor else None)
    in_names, out_names, out_avals = [], [], []
    for alloc in nc.m.functions[0].allocations:
        if not isinstance(alloc, mybir.MemoryLocationSet):
            continue
        name = alloc.memorylocations[0].name
        if alloc.kind == "ExternalInput":
            if name != partition_name:
                in_names.append(name)
        elif alloc.kind == "ExternalOutput":
            out_names.append(name)
            out_avals.append(jax.core.ShapedArray(
                tuple(alloc.tensor_shape), mybir.dt.np(alloc.dtype)))
    assert in_names == ["x"] and out_names == ["y"], (in_names, out_names)
    n_params = len(in_names)
    in_names_all = in_names + out_names
    if partition_name is not None:
        in_names_all.append(partition_name)
    donate = tuple(range(n_params, n_params + len(out_names)))

    def _body(*args):
        operands = list(args)
        if partition_name is not None:
            operands.append(b2j.partition_id_tensor())
        outs = b2j._bass_exec_p.bind(
            *operands,
            out_avals=tuple(out_avals),
            in_names=tuple(in_names_all),
            out_names=tuple(out_names),
            lowering_input_output_aliases=(),
            sim_require_finite=True,
            sim_require_nnan=True,
            nc=nc,
        )
        return tuple(outs)

    devices = jax.devices()[:NCORE]
    assert len(devices) == NCORE
    mesh = Mesh(np.asarray(devices), ("core",))
    nin = n_params + len(out_names)
    fn = jax.jit(
        shard_map(_body, mesh=mesh,
                  in_specs=(PartitionSpec("core"),) * nin,
                  out_specs=(PartitionSpec("core"),) * len(out_names),
                  check_rep=False),
        donate_argnums=donate,
        keep_unused=True,
    )
    return {"fn": fn, "prev": None}


def _assemble(res, y16):
    """Expand shipped [*,39] f16 block into final [*,51] f32 rows."""
    B = y16.astype(np.float32)
    res[:, 0:3] = 0.0                                   # pelvis
    res[:, 3:21] = B[:, 0:18]                           # j1..j6
    res[:, 24:30] = B[:, 18:24]                         # j8, j9
    res[:, 30:39] = B[:, 24:33]                         # j10, j11, j12
    res[:, 42:48] = B[:, 33:39]                         # j14, j15
    res[:, 21:24] = -B[:, 9:12]                         # rhip = -lhip
    res[:, 39:42] = 2.0 * B[:, 0:3] - B[:, 24:27]       # rsh = 2*torso - lsh
    res[:, 48:51] = 0.5 * (B[:, 15:18] + B[:, 18:21])   # thorax = (j6+j8)/2


def kernel(x: np.ndarray) -> np.ndarray:
    global _STATE
    if _STATE is None:
        _STATE = _init()
    st = _STATE

    x = np.asarray(x)
    if st["prev"] is None:
        st["prev"] = [np.zeros((NG, J), np.float16) for _ in range(NGRP)]

    # Dispatch group g, converting group g+1's input while g uploads.
    outs = []
    for g in range(NGRP):
        xg16 = x[g * NG:(g + 1) * NG].astype(np.float16)
        out, = st["fn"](xg16, st["prev"][g])
        outs.append(out)

    # Queue all device->host copies, then assemble each shard's rows while
    # later shards are still on the wire.
    res = np.empty((N, 51), np.float32)
    r0 = 0
    all_datas = []
    for out in outs:
        shards = sorted(out.addressable_shards,
                        key=lambda s: s.index[0].start or 0)
        datas = [s.data for s in shards]
        all_datas.extend(datas)
        for d in datas:
            try:
                d.copy_to_host_async()
            except Exception:
                pass
    for d in all_datas:
        y16 = np.asarray(d)
        r1 = r0 + y16.shape[0]
        _assemble(res[r0:r1], y16)
        r0 = r1
    assert r0 == N
    st["prev"] = outs                    # donate next call (already fetched)
    return res


# revision 24
# speedup vs baseline: 6.5580x; 1.3586x over previous
"""Trainium2 Bass kernel for nn_KinematicLayer: batched forward kinematics.

Full inputs x:[524288,26] f32 -> out:[524288,51] f32.

End-to-end wall time is dominated by the axon host<->device tunnel
(~50-80 MB/s, serial), so the kernel minimizes wire bytes:
  - x is uploaded as f16 ([N,26], 27 MB instead of 55 MB);
  - the device returns only the 13 non-derivable joint positions as f16
    ([N,39], 41 MB instead of 107 MB).  Host derives the rest:
    p0 = 0, p7 = -p4, p13 = 2*p1 - p10, thorax = (p6+p8)/2.
  - the jitted executable is cached across calls, and the previous call's
    (already fetched) device output buffers are donated back as the
    custom-call output operands, so no zero-buffer upload per call.

Device compute (per core: 65536 samples, 2 chunks of 128x256): per-sample
state tracked as (R 3x3, t 3); the five limb chains share one instruction
stream batched along the free dim (FD=1280).  Trig via half-angle
identities keeps every ACT Sin argument inside the spline's valid
[-pi,pi] range: u=sin(x/2), w=sin(x/4), v=1-2w^2=cos(x/2), cos=1-2u^2,
sin=2uv.  Intermediates fp16 (DVE 2x mode).
"""
import numpy as np
import jax
from jax.sharding import Mesh, PartitionSpec
from jax.experimental.shard_map import shard_map

import concourse.bass as bass
import concourse.tile as tile
import concourse.bass2jax as b2j
from concourse import bacc, mybir

AF = mybir.ActivationFunctionType
ALU = mybir.AluOpType
f32, f16, i8 = mybir.dt.float32, mybir.dt.float16, mybir.dt.int8

N, K = 524288, 26
J = 39                      # 13 joints x 3 shipped to host
NCORE = 8
NGRP = 2                    # pipeline groups (upload/exec/download overlap)
NG = N // NGRP              # 262144 samples per group
NPC = NG // NCORE           # 32768 samples per core per group
FD = min(256, NPC // 128)   # samples per partition per chunk
CHUNK = 128 * FD            # 32768 samples per chunk
NCHUNK = NPC // CHUNK       # 1

_S = np.array([300.0, 350.0, 75.0, 400.0, 73.96, 249.03, 250.0, 250.0, 170.0],
              np.float32) / 300.0
S0, S1, S2, S3, S4, S5, S6, S7, S8 = [float(v) for v in _S]

# chain order: (neck, Lleg, Rleg, Larm, Rarm); euler angle bases 5,9,13,17,21
DT1 = [S4, -S1, -S1, -S7, -S7]   # signed first-translation lengths
DT2 = [S5, -S0, -S0, -S6, -S6]   # signed distal-translation lengths

FDC = 5 * FD                 # batched chain free dim

# Output column layout: the 13 shipped joints in final joint order
# [1,2,3,4,5,6,8,9,10,11,12,14,15], 3 cols each — so host assembly is a
# handful of wide contiguous block copies.
#   cols  0: 3 j1 torso | 3: 6 j2 | 6: 9 j3 | 9:12 j4 | 12:15 j5 | 15:18 j6
#   cols 18:21 j8 | 21:24 j9 | 24:27 j10 | 27:30 j11 | 30:33 j12
#   cols 33:36 j14 | 36:39 j15
# Knee-level joints (2,5,8,11,14) land at col bases (3,12,18,27,33):
# chains {0,2,4} -> 3,18,33 (stride 15), chains {1,3} -> 12,27 (stride 15).
# Distal joints (3,6,9,12,15) at (6,15,21,30,36): same two-group split.
#
# int8 downlink: positions are exactly linear in the scale input x[:,25],
# so the device computes unit-scale positions (bounded per joint by its
# bone-length sum), quantizes q = v*127/(bound*MARGIN) to int8, and the
# host recovers v = q*(bound*MARGIN/127)*scale.
MARGIN = 1.02
B_TOR = S3                   # |torso| = S3 exactly
B_HIP = S2                   # |hip| = S2 exactly
B_SH = S3 + S8               # shoulder
BK = [S3 + S4, S2 + S1, S2 + S1, B_SH + S7, B_SH + S7]   # knee-level
BD = [BK[0] + S5, BK[1] + S0, BK[2] + S0, BK[3] + S6, BK[4] + S6]  # distal

# host dequant vector, col -> bound*MARGIN/127
_BOUNDS = ([B_TOR] * 3 + [BK[0]] * 3 + [BD[0]] * 3 + [B_HIP] * 3 +
           [BK[1]] * 3 + [BD[1]] * 3 + [BK[2]] * 3 + [BD[2]] * 3 +
           [B_SH] * 3 + [BK[3]] * 3 + [BD[3]] * 3 + [BK[4]] * 3 +
           [BD[4]] * 3)
DEQ = (np.asarray(_BOUNDS, np.float32) * MARGIN / 127.0)


def mk(ap, off, dims):
    """Custom free-dim AP on the same tile/tensor (keeps partition dim)."""
    return bass.AP(ap.tensor, ap.offset + off, [list(ap.ap[0])] + dims)


def build():
    nc = bacc.Bacc("TRN2", target_bir_lowering=False, debug=False,
                   num_devices=NCORE)
    x = nc.dram_tensor("x", [NPC, K], f16, kind="ExternalInput").ap()
    y = nc.dram_tensor("y", [NPC, J], i8, kind="ExternalOutput").ap()

    with tile.TileContext(nc) as tc:
        with (
            tc.tile_pool(name="io", bufs=1) as io,
            tc.tile_pool(name="per", bufs=1) as per,
            tc.tile_pool(name="scr", bufs=1) as scr,
        ):
            for ch in range(NCHUNK):
                build_chunk(nc, tc, io, per, scr, x, y, ch)
    nc.compile()
    return nc


def build_chunk(nc, tc, io, per, scr, x, y, ch):
    V, A = nc.vector, nc.scalar
    base = ch * CHUNK

    X = io.tile([128, K * FD], f16, tag="X")
    HX = K * FD // 2
    for h in range(2):
        nc.gpsimd.dma_start(X[:, h * HX:(h + 1) * HX],
                            bass.AP(x.tensor, base * K + h * HX,
                                    [[FD * K, 128], [1, HX]]))
    Y = io.tile([128, J * FD], i8, tag="Y")
    Xa = X[:]
    Ya = Y[:]

    def ycol(c):                       # output scalar col c strided [128,FD]
        return mk(Ya, c, [[J, FD]])

    def ygrpA(c0):                     # chains 0,2,4 -> 3 joints stride 15
        return mk(Ya, c0, [[15, 3], [J, FD]])

    def ygrpB(c0):                     # chains 1,3 -> 2 joints stride 15
        return mk(Ya, c0, [[15, 2], [J, FD]])

    def srcA(t):                       # chain-major [128,5*FD] -> chains 0,2,4
        a = t if isinstance(t, bass.AP) else t[:]
        return bass.AP(a.tensor, a.offset, [list(a.ap[0]), [2 * FD, 3], [1, FD]])

    def srcB(t):                       # chains 1,3
        a = t if isinstance(t, bass.AP) else t[:]
        return bass.AP(a.tensor, a.offset + FD,
                       [list(a.ap[0]), [2 * FD, 2], [1, FD]])

    # ---------------- trig: 5 groups ----------------
    def trig(tag, xap, n):
        fd = n * FD
        u = scr.tile([128, fd], f16, tag="trigU", name="trigU")
        w = scr.tile([128, fd], f16, tag="trigW", name="trigW")
        A.activation(u[:], xap, AF.Sin, scale=0.5)
        A.activation(w[:], xap, AF.Sin, scale=0.25)
        q = scr.tile([128, fd], f16, tag="trigQ", name="trigQ")
        c = per.tile([128, fd], f16, tag=f"C{tag}", name=f"C{tag}")
        s = per.tile([128, fd], f16, tag=f"S{tag}", name=f"S{tag}")
        A.square(q[:], u[:])
        V.tensor_scalar(c[:], q[:], -2.0, 1.0, ALU.mult, ALU.add)
        A.square(q[:], w[:])
        V.tensor_scalar(q[:], q[:], -2.0, 1.0, ALU.mult, ALU.add)  # v in q
        V.scalar_tensor_tensor(s[:], u[:], 2.0, q[:], ALU.mult, ALU.mult)
        return c, s

    Cpt, Spt = trig("pt", mk(Xa, 0, [[1, 5], [K, FD]]), 5)
    CS = [trig(f"p{j}", mk(Xa, 5 + j, [[4, 5], [K, FD]]), 5) for j in range(4)]

    def pt(t, i):
        return t[:, i * FD:(i + 1) * FD]

    c0, s0 = pt(Cpt, 0), pt(Spt, 0)
    c1, s1 = pt(Cpt, 1), pt(Spt, 1)
    c2, s2 = pt(Cpt, 2), pt(Spt, 2)
    c3, s3 = pt(Cpt, 3), pt(Spt, 3)
    c4, s4 = pt(Cpt, 4), pt(Spt, 4)

    def tt(out, a, b, op):
        V.tensor_tensor(out, a, b, op)

    def fresh(tag, fd=FD, dt=f16, pool=None):
        return (pool or scr).tile([128, fd], dt, tag=tag, name=tag)

    def mul(a, b, tag="m", fd=FD):
        o = fresh(tag, fd=fd)
        tt(o[:], a, b, ALU.mult)
        return o[:]

    def nmul(a, b, tag="m"):           # -(a*b)
        o = fresh(tag)
        V.scalar_tensor_tensor(o[:], a, -1.0, b, ALU.mult, ALU.mult)
        return o[:]

    def comb(a, b, op, tag="m", pool=None, fd=FD):
        o = fresh(tag, fd=fd, pool=pool)
        tt(o[:], a, b, op)
        return o[:]

    # ---------------- pelvis R ----------------
    ms0s1 = mul(s0, s1, "ms01")
    mc0s1 = mul(c0, s1, "mc01")
    P1x = nmul(s0, c1, "P1x")
    P1y = mul(c0, c1, "P1y")
    P1z = s1                                        # alias
    P0x = comb(mul(c0, c2), mul(ms0s1, s2, "m2"), ALU.subtract, "P0x", per)
    P0y = comb(mul(s0, c2), mul(mc0s1, s2, "m2"), ALU.add, "P0y", per)
    P0z = nmul(c1, s2, "P0z")
    P2x = comb(mul(c0, s2), mul(ms0s1, c2, "m2"), ALU.add, "P2x", per)
    P2y = comb(mul(s0, s2), mul(mc0s1, c2, "m2"), ALU.subtract, "P2y", per)
    P2z = mul(c1, c2, "P2z")
    P0 = (P0x, P0y, P0z)
    P1 = (P1x, P1y, P1z)
    P2 = (P2x, P2y, P2z)

    # ---------------- torso R = Rpel @ Rz3 @ Ry4 ----------------
    def colupd(cc, ss, A3, B3, tagp, pool=None, fd=FD):
        """returns cc*A + ss*B per component."""
        out = []
        for i, (a, b) in enumerate(zip(A3, B3)):
            out.append(comb(mul(cc, a, "ca", fd), mul(ss, b, "cb", fd), ALU.add,
                            f"{tagp}{i}", pool, fd))
        return tuple(out)

    def colupd_sub(cc, ss, A3, B3, tagp, pool=None, fd=FD):
        """returns cc*A - ss*B per component."""
        out = []
        for i, (a, b) in enumerate(zip(A3, B3)):
            out.append(comb(mul(cc, a, "ca", fd), mul(ss, b, "cb", fd),
                            ALU.subtract, f"{tagp}{i}", pool, fd))
        return tuple(out)

    D0t = colupd(c3, s3, P0, P1, "D0t")
    D1t = colupd_sub(c3, s3, P1, P0, "D1t", per)       # E1 = D1t
    E0 = colupd_sub(c4, s4, D0t, P2, "E0", per)
    E2 = colupd(s4, c4, D0t, P2, "E2", per)

    # ---------------- phase A translations (unit scale) ----------------
    TP = [per.tile([128, FDC], f16, tag=f"TP{c}", name=f"TP{c}")
          for c in range(3)]

    def tp_slice(c, i):
        return TP[c][:, i * FD:(i + 1) * FD]

    QTOR = 127.0 / MARGIN            # S3*127/(S3*MARGIN) folded
    QHIP = 127.0 / MARGIN
    QSH = 127.0 / (B_SH * MARGIN)
    for c in range(3):
        # torso t = S3*D1 -> Y joint1 (quantized) + TP[neck]
        A.mul(ycol(0 + c), D1t[c], QTOR)             # D1t*S3*127/(S3*M)
        A.mul(tp_slice(c, 0), D1t[c], S3)
        # hips: +-S2*P0 -> TP legs; left hip -> Y
        A.mul(tp_slice(c, 1), P0[c], S2)
        A.mul(tp_slice(c, 2), P0[c], -S2)
        A.mul(ycol(9 + c), P0[c], QHIP)              # P0*S2*127/(S2*M)
        # shoulders: t_tor +- S8*E0 -> TP arms; left shoulder -> Y
        u = fresh("shu")
        A.mul(u[:], E0[c], S8)
        tt(tp_slice(c, 3), tp_slice(c, 0), u[:], ALU.add)
        tt(tp_slice(c, 4), tp_slice(c, 0), u[:], ALU.subtract)
        A.mul(ycol(24 + c), tp_slice(c, 3), QSH)

    # ---------------- batched parent-R tiles ----------------
    # chains: 0=neck(E), 1,2=legs(P), 3,4=arms(E)
    PR = [[per.tile([128, FDC], f16, tag=f"PR{c}{i}", name=f"PR{c}{i}")
           for i in range(3)] for c in range(3)]
    for ci, (Ecol, Pcol) in enumerate(((E0, P0), (D1t, P1), (E2, P2))):
        for i in range(3):
            dst = PR[ci][i][:]
            e = Ecol[i]
            p = Pcol[i]

            def bc2(src):
                return bass.AP(src.tensor, src.offset,
                               [list(src.ap[0]), [0, 2], [1, FD]])

            A.copy(mk(dst, 0, [[1, FD]]), e)
            A.copy(mk(dst, FD, [[1, 2 * FD]]), bc2(p))
            A.copy(mk(dst, 3 * FD, [[1, 2 * FD]]), bc2(e))

    def prc(c):
        return tuple(PR[c][i][:] for i in range(3))

    cA, sA = (t[:] for t in CS[0])
    cB, sB = (t[:] for t in CS[1])
    cG, sG = (t[:] for t in CS[2])
    cD, sD = (t[:] for t in CS[3])

    # ---------------- batched chain (FD=1280 ops) ----------------
    bD0 = colupd(cA, sA, prc(0), prc(1), "bD0", per, FDC)
    bD1 = colupd_sub(cA, sA, prc(1), prc(0), "bD1", per, FDC)
    bK1 = colupd(cB, sB, bD1, prc(2), "bK1", per, FDC)
    bK2 = colupd_sub(cB, sB, prc(2), bD1, "bK2", per, FDC)
    bK2p = colupd(sG, cG, bD0, bK2, "bD1", per, FDC)  # reuse bD1 slots
    bC1 = colupd(cD, sD, bK1, bK2p, "bD0", per, FDC)  # reuse bD0 slots

    # constant tiles: per-chain signed bone lengths and quant scales
    dT1 = fresh("dT1", FDC, pool=per)
    dT2 = fresh("dT2", FDC, pool=per)
    Qk = fresh("Qk", FDC, pool=per)
    Qd = fresh("Qd", FDC, pool=per)
    for i in range(5):
        sl = slice(i * FD, (i + 1) * FD)
        V.memset(dT1[:, sl], DT1[i])
        V.memset(dT2[:, sl], DT2[i])
        V.memset(Qk[:, sl], 127.0 / (BK[i] * MARGIN))
        V.memset(Qd[:, sl], 127.0 / (BD[i] * MARGIN))

    for c in range(3):
        u = fresh("btr", FDC)
        tt(u[:], dT1[:], bK1[c], ALU.mult)
        kn = fresh("kn", FDC)
        tt(kn[:], TP[c][:], u[:], ALU.add)               # knee-level joints
        u2 = fresh("btr2", FDC)
        tt(u2[:], dT2[:], bC1[c], ALU.mult)
        ds = fresh("ds", FDC)
        tt(ds[:], kn[:], u2[:], ALU.add)                 # distal joints
        knq = fresh("knq", FDC)
        tt(knq[:], kn[:], Qk[:], ALU.mult)               # quantize
        dsq = fresh("dsq", FDC)
        tt(dsq[:], ds[:], Qd[:], ALU.mult)
        A.copy(ygrpA(3 + c), srcA(knq))
        A.copy(ygrpB(12 + c), srcB(knq))
        A.copy(ygrpA(6 + c), srcA(dsq))
        A.copy(ygrpB(15 + c), srcB(dsq))

    HY = J * FD // 2
    for h in range(2):
        nc.gpsimd.dma_start(bass.AP(y.tensor, base * J + h * HY,
                                    [[FD * J, 128], [1, HY]]),
                            Y[:, h * HY:(h + 1) * HY])


# ---------------------------------------------------------------------------
# Cached PJRT runner: jit(shard_map(bass_exec)) built once; the previous
# call's device output buffers (already copied to host) are donated back as
# the custom-call output operands, so steady-state wire traffic is just
# x (f16 up) + y (f16 down).
# ---------------------------------------------------------------------------
_STATE = None


def _init():
    nc = build()
    b2j.install_neuronx_cc_hook()

    partition_name = (nc.partition_id_tensor.name
                      if nc.partition_id_tensor else None)
    in_names, out_names, out_avals = [], [], []
    for alloc in nc.m.functions[0].allocations:
        if not isinstance(alloc, mybir.MemoryLocationSet):
            continue
        name = alloc.memorylocations[0].name
        if alloc.kind == "ExternalInput":
            if name != partition_name:
                in_names.append(name)
        elif alloc.kind == "ExternalOutput":
            out_names.append(name)
            out_avals.append(jax.core.ShapedArray(
                tuple(alloc.tensor_shape), mybir.dt.np(alloc.dtype)))
    assert in_names == ["x"] and out_names == ["y"], (in_names, out_names)
    n_params = len(in_names)
    in_names_all = in_names + out_names
    if partition_name is not None:
        in_names_all.append(partition_name)
    donate = tuple(range(n_params, n_params + len(out_names)))

    def _body(*args):
        operands = list(args)
        if partition_name is not None:
            operands.append(b2j.partition_id_tensor())
        outs = b2j._bass_exec_p.bind(
            *operands,
            out_avals=tuple(out_avals),
            in_names=tuple(in_names_all),
            out_names=tuple(out_names),
            lowering_input_output_aliases=(),
            sim_require_finite=True,
            sim_require_nnan=True,
            nc=nc,
        )
        return tuple(outs)

    devices = jax.devices()[:NCORE]
    assert len(devices) == NCORE
    mesh = Mesh(np.asarray(devices), ("core",))
    nin = n_params + len(out_names)
    fn = jax.jit(
        shard_map(_body, mesh=mesh,
                  in_specs=(PartitionSpec("core"),) * nin,
                  out_specs=(PartitionSpec("core"),) * len(out_names),
                  check_rep=False),
        donate_argnums=donate,
        keep_unused=True,
    )
    return {"fn": fn, "prev": None}


def _assemble(res, y8, scl):
    """Dequantize shipped [*,39] int8 block into final [*,51] f32 rows."""
    B = y8.astype(np.float32)
    B *= DEQ[None, :]
    B *= scl[:, None]
    res[:, 0:3] = 0.0                                   # pelvis
    res[:, 3:21] = B[:, 0:18]                           # j1..j6
    res[:, 24:30] = B[:, 18:24]                         # j8, j9
    res[:, 30:39] = B[:, 24:33]                         # j10, j11, j12
    res[:, 42:48] = B[:, 33:39]                         # j14, j15
    res[:, 21:24] = -B[:, 9:12]                         # rhip = -lhip
    res[:, 39:42] = 2.0 * B[:, 0:3] - B[:, 24:27]       # rsh = 2*torso - lsh
    res[:, 48:51] = 0.5 * (B[:, 15:18] + B[:, 18:21])   # thorax = (j6+j8)/2


def kernel(x: np.ndarray) -> np.ndarray:
    global _STATE
    if _STATE is None:
        _STATE = _init()
    st = _STATE

    x = np.asarray(x)
    scl = np.ascontiguousarray(x[:, 25], dtype=np.float32)
    if st["prev"] is None:
        st["prev"] = [np.zeros((NG, J), np.int8) for _ in range(NGRP)]

    # Dispatch group g, converting group g+1's input while g uploads and
    # queueing g's device->host copies right away.
    outs = []
    all_datas = []
    for g in range(NGRP):
        xg16 = x[g * NG:(g + 1) * NG].astype(np.float16)
        out, = st["fn"](xg16, st["prev"][g])
        outs.append(out)
        shards = sorted(out.addressable_shards,
                        key=lambda s: s.index[0].start or 0)
        datas = [s.data for s in shards]
        all_datas.extend(datas)
        for d in datas:
            try:
                d.copy_to_host_async()
            except Exception:
                pass

    # Assemble each shard's rows while later shards are still on the wire.
    res = np.empty((N, 51), np.float32)
    r0 = 0
    for d in all_datas:
        y8 = np.asarray(d)
        r1 = r0 + y8.shape[0]
        _assemble(res[r0:r1], y8, scl[r0:r1])
        r0 = r1
    assert r0 == N
    st["prev"] = outs                    # donate next call (already fetched)
    return res
